# revision 63
# baseline (speedup 1.0000x reference)
"""CensusLoss Trainium2 kernel (v2).

Census transform loss: grayscale -> 48 shifted binary comparisons (7x7 patch,
reflect pad 3) -> mean |pred_census - target_census|.

Sharding: pure data parallel, batch dim B=8 across 8 NeuronCores (one image
per core). Host combines exact per-core partial sums.

Key ideas vs v1:
  * Antisymmetry: cmp_{-d}(x) = NOT cmp_d(x-d) except at ties/borders, so
    XOR_{-d}(x) = XOR_d(x-d) and sum(XOR_{-d}) ~= sum(XOR_d). Compute the 24
    offsets with (di>0) or (di==0, dj>0) and double the result (validated
    1.4e-5 rel err on the reference input distribution).
  * Paired bands: one [128, 2*3640] tile holds pred|target bands with equal
    layout, so ONE DVE is_gt (2x mode) produces both images' cmp maps.
  * Three per-offset reduction routes balance all four engines:
      px:   Pool computes xor = cmpP != cmpT (fp8 out), PE sums it with
            fp8 DoubleRow ones-matmuls into a [1,512] PSUM accumulator.
      gram: PE computes gram(cmpP, cmpT) diag (bf16 matmuls) + paired ones
            sums; XOR = sum(cP) + sum(cT) - 2*sum(cP*cT).
      sign: PE computes diff = center - neighbor via +I/-I matmuls into
            PSUM, ACT binarizes with Sign (fp8 +-1 maps), PE reduces with
            fp8 DoubleRow gram; XOR = (N - sum(sP*sT))/2 (ties -> 1/2,
            unbiased vs the strict-gt reference).
"""

import numpy as np

B, C, H, W = 8, 3, 512, 512
N_CORES = 8
PAD = 3
Wp = 520            # padded row width (518 used + 2 spare)
COL0 = 4            # tile col of gray col 0 (even => 4B-aligned in bf16)
RPP = 4             # gray rows per partition (512 / 128)
SLOTS = RPP + PAD   # 7: 4 center rows + 3 halo-below rows
SEG = SLOTS * Wp    # 3640 elements per image segment
FREE = RPP * W      # 2048
PAIR = 2 * FREE     # 4096
NPIX = H * W        # 262144

_CACHE = {}


def _offsets():
    # halved offset set: di>0, or di==0 and dj>0; di==0 first (those skip
    # the halo-row dependency, so their cmps start right at gray-done)
    di0, rest = [], []
    for di in range(0, PAD + 1):
        for dj in range(-PAD, PAD + 1):
            if di == 0 and dj <= 0:
                continue
            (di0 if di == 0 else rest).append((di, dj))
    return di0 + rest


def _routes(n_off):
    """Assign each offset a route:
      p1:   DVE cmp + Pool d=cP-cT (fp8) + PE fp8 self-gram (XOR = sum d^2)
      sign: PE +-I diffs -> ACT Sign (fp8 +-1) -> PE fp8 gram
      a1:   DVE cmp + DVE xor + PE bf16 ones-sums
      gram: DVE cmp + PE bf16 gram + pair-sums
    """
    n_sign = int(_CACHE.get("n_sign", 7))
    n_gram = int(_CACHE.get("n_gram", 0))
    n_a1 = int(_CACHE.get("n_a1", 7))
    n_p1 = n_off - n_sign - n_gram - n_a1
    # proportional interleave so every engine's stream stays dense; end on
    # a1 (short DVE-only chains) to keep the tail off Pool/ACT
    rem = {"p1": n_p1, "sign": n_sign, "a1": max(n_a1 - 2, 0),
           "gram": n_gram}
    tot = sum(rem.values())
    routes = []
    acc = {k: 0.0 for k in rem}
    for _ in range(tot):
        for k in rem:
            acc[k] += rem[k] / tot
        k = max(acc, key=lambda k: (acc[k], k == "p1"))
        acc[k] -= 1.0
        routes.append(k)
    routes += ["a1"] * (n_a1 - max(n_a1 - 2, 0))
    return routes


def _build_bass(n_off=24):
    from concourse import bacc, mybir
    from concourse.ap import AP
    from concourse.tile import TileContext
    from concourse.alu_op_type import AluOpType as op

    dt = mybir.dt
    AF = mybir.ActivationFunctionType
    nc = bacc.Bacc("TRN2", debug=False)

    pred = nc.dram_tensor("pred", [C, H, W], dt.float32, kind="ExternalInput")
    target = nc.dram_tensor("target", [C, H, W], dt.float32,
                            kind="ExternalInput")
    sums_out = nc.dram_tensor("sums_out", [1, 512], dt.float32,
                              kind="ExternalOutput")
    gram01_out = nc.dram_tensor("gram01_out", [128, 128], dt.float32,
                                kind="ExternalOutput")
    sgram_out = nc.dram_tensor("sgram_out", [128, 128], dt.float32,
                               kind="ExternalOutput")
    dgram_out = nc.dram_tensor("dgram_out", [128, 128], dt.float32,
                               kind="ExternalOutput")

    offs = _offsets()[:n_off]
    routes = _routes(len(offs))

    with TileContext(nc) as tc:
      with tc.tile_pool(name="sbuf", bufs=1) as pool:
        # ---- constants ----
        ones_bf = pool.tile([128, 2], dt.bfloat16, name="ones_bf",
                            tag="ones_bf")
        nc.vector.memset(ones_bf, 1.0)

        colidx = pool.tile([128, 128], dt.int32, name="colidx", tag="colidx")
        nc.gpsimd.iota(colidx, pattern=[[1, 128]], base=0,
                       channel_multiplier=0)
        rowidx_i = pool.tile([128, 1], dt.int32, name="rowidx_i",
                             tag="rowidx_i")
        nc.gpsimd.iota(rowidx_i, pattern=[[0, 1]], base=0,
                       channel_multiplier=1)
        rowidx = pool.tile([128, 1], dt.float32, name="rowidx", tag="rowidx")
        nc.vector.tensor_copy(out=rowidx, in_=rowidx_i)
        ident_p = pool.tile([128, 128], dt.bfloat16, name="ident_p",
                            tag="ident_p")
        nc.vector.tensor_scalar(out=ident_p, in0=colidx, scalar1=rowidx,
                                scalar2=None, op0=op.is_equal)
        ident_n = pool.tile([128, 128], dt.bfloat16, name="ident_n",
                            tag="ident_n")
        nc.vector.tensor_scalar(out=ident_n, in0=colidx, scalar1=rowidx,
                                scalar2=-1.0, op0=op.is_equal, op1=op.mult)

        # ---- input loads: channels interleaved on two queues ----
        # single queue, strict order: P channels (whole) first so the
        # pred-side band is ready at ~half the load time; T channels split
        # into two half-loads (rows-pairs) so T-gray pipelines with the
        # transfers and the T band closes right after the last byte lands
        chs = {}
        for nm, c in [("p", 0), ("p", 1), ("p", 2),
                      ("t", 0), ("t", 1), ("t", 2)]:
            chs[(nm, c)] = pool.tile([128, FREE], dt.float32,
                                     name=f"ch_{nm}{c}", tag=f"ch_{nm}{c}",
                                     bufs=1)
        for nm, src in (("p", pred), ("t", target)):
            for h in range(2):
                for c in range(3):
                    v = src.ap()[c].rearrange("(p r) w -> p (r w)", p=128)
                    sl = slice(h * 1024, (h + 1) * 1024)
                    nc.sync.dma_start(out=chs[(nm, c)][:, sl], in_=v[:, sl])

        # ---- paired band tile: [P seg | T seg], 7 slots x 520 each ----
        bandA = pool.tile([128, 2 * SEG], dt.bfloat16, name="bandA",
                          tag="bandA")

        hp = tc.high_priority()
        hp.__enter__()
        segv = bandA.rearrange("p (s r w) -> p s r w", s=2, w=Wp)
        for si, nm in enumerate(("p", "t")):
            ch = [chs[(nm, c)] for c in range(3)]
            # zero spare cols (0 and 519) of the 4 center slots
            nc.vector.memset(
                AP(bandA.tensor, bandA.offset + si * SEG,
                   [[bandA.ap[0][0], 128], [Wp, RPP], [Wp - 1, 2]]),
                0.0)
            padv = segv[:, si, 0:RPP, :]
            halves = (0, 1)
            for h in halves:
                sl = slice(0, FREE) if h is None else slice(h * 1024,
                                                            (h + 1) * 1024)
                n = sl.stop - sl.start
                g1 = pool.tile([128, n], dt.bfloat16, name=f"g1_{nm}{h}",
                               tag=f"g1{h}", bufs=1)
                nc.scalar.mul(g1, ch[0][:, sl], 0.299)
                gb = pool.tile([128, n], dt.bfloat16, name=f"gb_{nm}{h}",
                               tag=f"gb{h}", bufs=1)
                nc.scalar.mul(gb, ch[1][:, sl], 0.587)
                g2 = pool.tile([128, n], dt.bfloat16, name=f"g2_{nm}{h}",
                               tag=f"g2{h}", bufs=1)
                # P-side adds run on the otherwise-idle Pool head window,
                # keeping DVE free for the comparison stream
                geng = nc.gpsimd if nm == "p" else nc.vector
                geng.tensor_tensor(out=g2, in0=g1, in1=gb, op=op.add)
                g2v = g2.rearrange("p (r w) -> p r w", w=W)
                r0 = 0 if h in (None, 0) else 2
                rn = RPP if h is None else 2
                outv = padv[:, r0:r0 + rn, COL0:COL0 + W]
                if nm == "t":
                    # fused (B * 0.114) + g2 straight into the band slots:
                    # shortest serial chain after each T half-load lands
                    ch2v = ch[2][:, sl].rearrange("p (r w) -> p r w", w=W)
                    nc.vector.scalar_tensor_tensor(
                        out=outv, in0=ch2v, scalar=0.114,
                        in1=g2v, op0=op.mult, op1=op.add)
                else:
                    gc = pool.tile([128, n], dt.bfloat16, name=f"gc_{nm}{h}",
                                   tag=f"gc{h}", bufs=1)
                    nc.scalar.mul(gc, ch[2][:, sl], 0.114)
                    gcv = gc.rearrange("p (r w) -> p r w", w=W)
                    nc.gpsimd.tensor_tensor(out=outv, in0=g2v, in1=gcv,
                                            op=op.add)
                # reflect cols for this half's rows: col COL0-t = gray col t
                hv = padv[:, r0:r0 + rn, :]
                ghv = hv[:, :, COL0:COL0 + W]
                nc.vector.tensor_copy(out=hv[:, :, 1:4],
                                      in_=ghv[:, :, 3:0:-1])
                nc.vector.tensor_copy(out=hv[:, :, 516:519],
                                      in_=ghv[:, :, 510:507:-1])

        # ---- halos (bottom 3 slots per segment), all SBUF->SBUF ----
        pstride = bandA.ap[0][0]
        for si, q in ((0, nc.sync), (1, nc.sync)):
            base = bandA.offset + si * SEG
            # band[p][slots 4..6] <- band[p+1][slots 0..2], p = 0..126
            q.dma_start(
                out=AP(bandA.tensor, base + RPP * Wp,
                       [[pstride, 127], [1, PAD * Wp]]),
                in_=AP(bandA.tensor, base + pstride,
                       [[pstride, 127], [1, PAD * Wp]]))
            # reflect edge: band[127][slot 4+t] = band[127][slot 2-t]
            q.dma_start(
                out=AP(bandA.tensor, base + 127 * pstride + RPP * Wp,
                       [[pstride, 1], [Wp, PAD], [1, Wp]]),
                in_=AP(bandA.tensor, base + 127 * pstride + 2 * Wp,
                       [[pstride, 1], [-Wp, PAD], [1, Wp]]))

        hp.__exit__(None, None, None)

        segA = bandA.rearrange("p (s r w) -> p s r w", s=2, w=Wp)
        center = segA[:, :, 0:RPP, COL0:COL0 + W]

        def nb_view(di, dj):
            return segA[:, :, di:di + RPP, COL0 + dj:COL0 + dj + W]

        sign_list = [i for i, r in enumerate(routes) if r == "sign"]
        gram_list = [i for i, r in enumerate(routes) if r == "gram"]
        p1_list = [i for i, r in enumerate(routes) if r == "p1"]
        a1_list = [i for i, r in enumerate(routes) if r == "a1"]
        n_sign = len(sign_list)

        with tc.tile_pool(name="psum", bufs=1, space="PSUM") as ppool:
            sums = ppool.tile([1, 512], dt.float32, name="sums")
            gram01 = (ppool.tile([128, 128], dt.float32, name="gram01")
                      if gram_list else None)
            sgram = ppool.tile([128, 128], dt.float32, name="sgram")
            dgram = ppool.tile([128, 128], dt.float32, name="dgram")

            sums_first = True
            g01_first = True
            sg_first = True
            dg_first = True
            # precompute which emission is the last contributor per accum
            n_sums_mm = 4 * len(a1_list) + 8 * len(gram_list)
            n_g01_mm = 16 * len(gram_list)
            n_sg_mm = 8 * len(sign_list)
            n_dg_mm = 8 * len(p1_list)
            sums_done = 0
            g01_done = 0
            sg_done = 0
            dg_done = 0

            for i, (di, dj) in enumerate(offs):
                r = routes[i]
                if r == "sign":
                    nb = nb_view(di, dj)
                    sm = {}
                    for si, nm in enumerate(("p", "t")):
                        smt = pool.tile([128, FREE], dt.float8e4,
                                        name=f"sm_{nm}_{i}", tag=f"sm_{nm}",
                                        bufs=2)
                        for h in range(2):
                            dps = ppool.tile([128, 1024], dt.float32,
                                             name=f"dps_{i}_{nm}{h}",
                                             tag="dps", bufs=2)
                            # group by lhsT (+I then -I) to halve ldweights
                            for q in range(2):
                                rr = 2 * h + q
                                nc.tensor.matmul(
                                    dps[:, q * 512:(q + 1) * 512], ident_p,
                                    center[:, si, rr:rr + 1, :],
                                    start=True, stop=False,
                                    skip_group_check=True)
                            for q in range(2):
                                rr = 2 * h + q
                                nc.tensor.matmul(
                                    dps[:, q * 512:(q + 1) * 512], ident_n,
                                    nb[:, si, rr:rr + 1, :],
                                    start=False, stop=True,
                                    skip_group_check=True)
                            nc.scalar.activation(
                                out=smt[:, h * 1024:(h + 1) * 1024],
                                in_=dps, func=AF.Sign)
                        sm[nm] = smt
                    smpv = sm["p"].rearrange("p (b j m) -> p b j m", j=2,
                                             m=128)
                    smtv = sm["t"].rearrange("p (b j m) -> p b j m", j=2,
                                             m=128)
                    for b in range(8):
                        sg_done += 1
                        nc.tensor.matmul(
                            sgram[:, :],
                            smpv[:, b], smtv[:, b],
                            start=sg_first, stop=(sg_done == n_sg_mm),
                            perf_mode=mybir.MatmulPerfMode.DoubleRow,
                            skip_group_check=True)
                        sg_first = False
                    continue

                # cmp maps; the first few offsets split per-image so the
                # pred side runs while target channels are still loading
                cmp = pool.tile([128, PAIR], dt.bfloat16,
                                name=f"cmp_{i}", tag=f"cmp_{r}",
                                bufs=(5 if r == "p1" else 3))
                cv = cmp.rearrange("p (s r w) -> p s r w", s=2, w=W)
                nbv = nb_view(di, dj)
                if di == 0 and i < 3:
                    # di==0 rows r only read band slots r: the top half is
                    # ready right after the first half-gray of each image
                    for si in range(2):
                        for r0, r1 in ((0, 2), (2, RPP)):
                            nc.vector.tensor_tensor(
                                out=cv[:, si:si + 1, r0:r1],
                                in0=center[:, si:si + 1, r0:r1],
                                in1=nbv[:, si:si + 1, r0:r1],
                                op=op.is_gt)
                elif i < 10:
                    for si in range(2):
                        nc.vector.tensor_tensor(out=cv[:, si:si + 1],
                                                in0=center[:, si:si + 1],
                                                in1=nbv[:, si:si + 1],
                                                op=op.is_gt)
                else:
                    nc.vector.tensor_tensor(out=cv, in0=center, in1=nbv,
                                            op=op.is_gt)

                if r == "p1":
                    # d = cmpP - cmpT in {-1,0,1} (fp8 exact); XOR = sum(d^2)
                    dd = pool.tile([128, FREE], dt.float8e4,
                                   name=f"dd_{i}", tag="dd", bufs=4)
                    nc.gpsimd.tensor_tensor(out=dd, in0=cmp[:, 0:FREE],
                                            in1=cmp[:, FREE:PAIR],
                                            op=op.subtract)
                    ddv = dd.rearrange("p (b j m) -> p b j m", j=2, m=128)
                    for b in range(8):
                        dg_done += 1
                        nc.tensor.matmul(
                            dgram[:, :], ddv[:, b], ddv[:, b],
                            start=dg_first, stop=(dg_done == n_dg_mm),
                            perf_mode=mybir.MatmulPerfMode.DoubleRow,
                            skip_group_check=True)
                        dg_first = False
                elif r == "a1":
                    xo = pool.tile([128, FREE], dt.bfloat16,
                                   name=f"xo_{i}", tag="xo", bufs=4)
                    nc.vector.tensor_tensor(out=xo, in0=cmp[:, 0:FREE],
                                            in1=cmp[:, FREE:PAIR],
                                            op=op.not_equal)
                    for k in range(4):
                        sums_done += 1
                        nc.tensor.matmul(
                            sums[0:1, :], ones_bf[:, 0:1],
                            xo[:, k * 512:(k + 1) * 512],
                            start=sums_first, stop=(sums_done == n_sums_mm),
                            skip_group_check=True)
                        sums_first = False
                else:  # gram
                    for b in range(16):
                        g01_done += 1
                        nc.tensor.matmul(
                            gram01[:, :],
                            cmp[:, b * 128:(b + 1) * 128],
                            cmp[:, FREE + b * 128:FREE + (b + 1) * 128],
                            start=g01_first, stop=(g01_done == n_g01_mm),
                            skip_group_check=True)
                        g01_first = False
                    for k in range(8):
                        sums_done += 1
                        nc.tensor.matmul(
                            sums[0:1, :], ones_bf[:, 0:1],
                            cmp[:, k * 512:(k + 1) * 512],
                            start=sums_first, stop=(sums_done == n_sums_mm),
                            skip_group_check=True)
                        sums_first = False

            # ---- drain accumulators to SBUF and DMA out ----
            sums_sb = pool.tile([1, 512], dt.float32, name="sums_sb",
                                tag="sums_sb")
            g01_sb = pool.tile([128, 128], dt.float32, name="g01_sb",
                               tag="g01_sb")
            sg_sb = pool.tile([128, 128], dt.float32, name="sg_sb",
                              tag="sg_sb")
            dg_sb = pool.tile([128, 128], dt.float32, name="dg_sb",
                              tag="dg_sb")
            # drain order: accumulators that stop earliest first, so their
            # output DMAs don't queue behind the late-stopping sums
            if n_dg_mm:
                nc.vector.tensor_copy(out=dg_sb, in_=dgram)
                nc.sync.dma_start(out=dgram_out.ap(), in_=dg_sb)
            if n_sg_mm:
                nc.vector.tensor_copy(out=sg_sb, in_=sgram)
                nc.sync.dma_start(out=sgram_out.ap(), in_=sg_sb)
            if n_g01_mm:
                nc.vector.tensor_copy(out=g01_sb, in_=gram01)
                nc.sync.dma_start(out=gram01_out.ap(), in_=g01_sb)
            if n_sums_mm:
                nc.vector.tensor_copy(out=sums_sb, in_=sums)
                nc.sync.dma_start(out=sums_out.ap(), in_=sums_sb)
            nc._use_outs = (n_sums_mm > 0, n_g01_mm > 0, n_sg_mm > 0,
                            n_dg_mm > 0)

    nc.finalize()
    nc._n_sign = n_sign
    return nc


def kernel(pred: np.ndarray, target: np.ndarray) -> np.ndarray:
    from concourse import bass_utils

    if "nc" not in _CACHE:
        _CACHE["nc"] = _build_bass()
    nc = _CACHE["nc"]
    n_sign = nc._n_sign

    pred = np.ascontiguousarray(pred, dtype=np.float32)
    target = np.ascontiguousarray(target, dtype=np.float32)
    in_maps = [
        {"pred": pred[b], "target": target[b]} for b in range(N_CORES)
    ]
    res = bass_utils.run_bass_kernel_spmd(nc, in_maps,
                                          core_ids=list(range(N_CORES)))
    use_sums, use_g01, use_sg, use_dg = nc._use_outs
    total = 0.0
    for r in res.results:
        if use_sums:
            total += float(r["sums_out"].astype(np.float64).sum())
        if use_g01:
            total -= 2.0 * float(
                np.diag(r["gram01_out"]).astype(np.float64).sum())
        if use_dg:
            total += float(np.diag(r["dgram_out"]).astype(np.float64).sum())
        if use_sg:
            total += (n_sign * NPIX - float(
                np.diag(r["sgram_out"]).astype(np.float64).sum())) / 2.0
    mean = 2.0 * total / (B * 48 * H * W)
    return np.array(mean, dtype=np.float32)


# revision 65
# speedup vs baseline: 1.0550x; 1.0550x over previous
"""CensusLoss Trainium2 kernel (v2).

Census transform loss: grayscale -> 48 shifted binary comparisons (7x7 patch,
reflect pad 3) -> mean |pred_census - target_census|.

Sharding: pure data parallel, batch dim B=8 across 8 NeuronCores (one image
per core). Host combines exact per-core partial sums.

Key ideas vs v1:
  * Antisymmetry: cmp_{-d}(x) = NOT cmp_d(x-d) except at ties/borders, so
    XOR_{-d}(x) = XOR_d(x-d) and sum(XOR_{-d}) ~= sum(XOR_d). Compute the 24
    offsets with (di>0) or (di==0, dj>0) and double the result (validated
    1.4e-5 rel err on the reference input distribution).
  * Paired bands: one [128, 2*3640] tile holds pred|target bands with equal
    layout, so ONE DVE is_gt (2x mode) produces both images' cmp maps.
  * Three per-offset reduction routes balance all four engines:
      px:   Pool computes xor = cmpP != cmpT (fp8 out), PE sums it with
            fp8 DoubleRow ones-matmuls into a [1,512] PSUM accumulator.
      gram: PE computes gram(cmpP, cmpT) diag (bf16 matmuls) + paired ones
            sums; XOR = sum(cP) + sum(cT) - 2*sum(cP*cT).
      sign: PE computes diff = center - neighbor via +I/-I matmuls into
            PSUM, ACT binarizes with Sign (fp8 +-1 maps), PE reduces with
            fp8 DoubleRow gram; XOR = (N - sum(sP*sT))/2 (ties -> 1/2,
            unbiased vs the strict-gt reference).
"""

import numpy as np

B, C, H, W = 8, 3, 512, 512
N_CORES = 8
PAD = 3
Wp = 520            # padded row width (518 used + 2 spare)
COL0 = 4            # tile col of gray col 0 (even => 4B-aligned in bf16)
RPP = 4             # gray rows per partition (512 / 128)
SLOTS = RPP + PAD   # 7: 4 center rows + 3 halo-below rows
SEG = SLOTS * Wp    # 3640 elements per image segment
FREE = RPP * W      # 2048
PAIR = 2 * FREE     # 4096
NPIX = H * W        # 262144

_CACHE = {}


def _offsets():
    # halved offset set: di>0, or di==0 and dj>0; di==0 first (those skip
    # the halo-row dependency, so their cmps start right at gray-done)
    di0, rest = [], []
    for di in range(0, PAD + 1):
        for dj in range(-PAD, PAD + 1):
            if di == 0 and dj <= 0:
                continue
            (di0 if di == 0 else rest).append((di, dj))
    return di0 + rest


def _routes(n_off):
    """Assign each offset a route:
      p1:   DVE cmp + Pool d=cP-cT (fp8) + PE fp8 self-gram (XOR = sum d^2)
      sign: PE +-I diffs -> ACT Sign (fp8 +-1) -> PE fp8 gram
      a1:   DVE cmp + DVE xor + PE bf16 ones-sums
      gram: DVE cmp + PE bf16 gram + pair-sums
    """
    n_sign = int(_CACHE.get("n_sign", 7))
    n_gram = int(_CACHE.get("n_gram", 0))
    n_a1 = int(_CACHE.get("n_a1", 7))
    n_p1 = n_off - n_sign - n_gram - n_a1
    # proportional interleave so every engine's stream stays dense; end on
    # a1 (short DVE-only chains) to keep the tail off Pool/ACT
    rem = {"p1": n_p1, "sign": n_sign, "a1": max(n_a1 - 2, 0),
           "gram": n_gram}
    tot = sum(rem.values())
    routes = []
    acc = {k: 0.0 for k in rem}
    for _ in range(tot):
        for k in rem:
            acc[k] += rem[k] / tot
        k = max(acc, key=lambda k: (acc[k], k == "p1"))
        acc[k] -= 1.0
        routes.append(k)
    routes += ["a1"] * (n_a1 - max(n_a1 - 2, 0))
    return routes


def _build_bass(n_off=24):
    from concourse import bacc, mybir
    from concourse.ap import AP
    from concourse.tile import TileContext
    from concourse.alu_op_type import AluOpType as op

    dt = mybir.dt
    AF = mybir.ActivationFunctionType
    nc = bacc.Bacc("TRN2", debug=False)

    pred = nc.dram_tensor("pred", [C, H, W], dt.float32, kind="ExternalInput")
    target = nc.dram_tensor("target", [C, H, W], dt.float32,
                            kind="ExternalInput")
    sums_out = nc.dram_tensor("sums_out", [1, 512], dt.float32,
                              kind="ExternalOutput")
    gram01_out = nc.dram_tensor("gram01_out", [128, 128], dt.float32,
                                kind="ExternalOutput")
    sgram_out = nc.dram_tensor("sgram_out", [128, 128], dt.float32,
                               kind="ExternalOutput")
    dgram_out = nc.dram_tensor("dgram_out", [128, 128], dt.float32,
                               kind="ExternalOutput")

    offs = _offsets()[:n_off]
    routes = _routes(len(offs))

    with TileContext(nc) as tc:
      with tc.tile_pool(name="sbuf", bufs=1) as pool:
        # ---- constants ----
        ones_bf = pool.tile([128, 2], dt.bfloat16, name="ones_bf",
                            tag="ones_bf")
        nc.vector.memset(ones_bf, 1.0)

        colidx = pool.tile([128, 128], dt.int32, name="colidx", tag="colidx")
        nc.gpsimd.iota(colidx, pattern=[[1, 128]], base=0,
                       channel_multiplier=0)
        rowidx_i = pool.tile([128, 1], dt.int32, name="rowidx_i",
                             tag="rowidx_i")
        nc.gpsimd.iota(rowidx_i, pattern=[[0, 1]], base=0,
                       channel_multiplier=1)
        rowidx = pool.tile([128, 1], dt.float32, name="rowidx", tag="rowidx")
        nc.vector.tensor_copy(out=rowidx, in_=rowidx_i)
        ident_p = pool.tile([128, 128], dt.bfloat16, name="ident_p",
                            tag="ident_p")
        nc.vector.tensor_scalar(out=ident_p, in0=colidx, scalar1=rowidx,
                                scalar2=None, op0=op.is_equal)
        ident_n = pool.tile([128, 128], dt.bfloat16, name="ident_n",
                            tag="ident_n")
        nc.vector.tensor_scalar(out=ident_n, in0=colidx, scalar1=rowidx,
                                scalar2=-1.0, op0=op.is_equal, op1=op.mult)

        # ---- input loads: channels interleaved on two queues ----
        # single queue, strict order: P channels (whole) first so the
        # pred-side band is ready at ~half the load time; T channels split
        # into two half-loads (rows-pairs) so T-gray pipelines with the
        # transfers and the T band closes right after the last byte lands
        chs = {}
        for nm, c in [("p", 0), ("p", 1), ("p", 2),
                      ("t", 0), ("t", 1), ("t", 2)]:
            chs[(nm, c)] = pool.tile([128, FREE], dt.float32,
                                     name=f"ch_{nm}{c}", tag=f"ch_{nm}{c}",
                                     bufs=1)
        for nm, src in (("p", pred), ("t", target)):
            for h in range(2):
                for c in range(3):
                    v = src.ap()[c].rearrange("(p r) w -> p (r w)", p=128)
                    sl = slice(h * 1024, (h + 1) * 1024)
                    nc.sync.dma_start(out=chs[(nm, c)][:, sl], in_=v[:, sl])

        # ---- paired band tile: [P seg | T seg], 7 slots x 520 each ----
        bandA = pool.tile([128, 2 * SEG], dt.bfloat16, name="bandA",
                          tag="bandA")

        hp = tc.high_priority()
        hp.__enter__()
        segv = bandA.rearrange("p (s r w) -> p s r w", s=2, w=Wp)
        for si, nm in enumerate(("p", "t")):
            ch = [chs[(nm, c)] for c in range(3)]
            # zero spare cols (0 and 519) of the 4 center slots
            nc.vector.memset(
                AP(bandA.tensor, bandA.offset + si * SEG,
                   [[bandA.ap[0][0], 128], [Wp, RPP], [Wp - 1, 2]]),
                0.0)
            padv = segv[:, si, 0:RPP, :]
            halves = (0, 1)
            for h in halves:
                sl = slice(0, FREE) if h is None else slice(h * 1024,
                                                            (h + 1) * 1024)
                n = sl.stop - sl.start
                g1 = pool.tile([128, n], dt.bfloat16, name=f"g1_{nm}{h}",
                               tag=f"g1{h}", bufs=1)
                nc.scalar.mul(g1, ch[0][:, sl], 0.299)
                gb = pool.tile([128, n], dt.bfloat16, name=f"gb_{nm}{h}",
                               tag=f"gb{h}", bufs=1)
                nc.scalar.mul(gb, ch[1][:, sl], 0.587)
                g2 = pool.tile([128, n], dt.bfloat16, name=f"g2_{nm}{h}",
                               tag=f"g2{h}", bufs=1)
                nc.vector.tensor_add(g2, g1, gb)
                g2v = g2.rearrange("p (r w) -> p r w", w=W)
                r0 = 0 if h in (None, 0) else 2
                rn = RPP if h is None else 2
                outv = padv[:, r0:r0 + rn, COL0:COL0 + W]
                if nm == "t":
                    # fused (B * 0.114) + g2 straight into the band slots:
                    # shortest serial chain after each T half-load lands
                    ch2v = ch[2][:, sl].rearrange("p (r w) -> p r w", w=W)
                    nc.vector.scalar_tensor_tensor(
                        out=outv, in0=ch2v, scalar=0.114,
                        in1=g2v, op0=op.mult, op1=op.add)
                else:
                    gc = pool.tile([128, n], dt.bfloat16, name=f"gc_{nm}{h}",
                                   tag=f"gc{h}", bufs=1)
                    nc.scalar.mul(gc, ch[2][:, sl], 0.114)
                    gcv = gc.rearrange("p (r w) -> p r w", w=W)
                    nc.vector.tensor_tensor(out=outv, in0=g2v, in1=gcv,
                                            op=op.add)
                # reflect cols for this half's rows: col COL0-t = gray col t
                hv = padv[:, r0:r0 + rn, :]
                ghv = hv[:, :, COL0:COL0 + W]
                nc.vector.tensor_copy(out=hv[:, :, 1:4],
                                      in_=ghv[:, :, 3:0:-1])
                nc.vector.tensor_copy(out=hv[:, :, 516:519],
                                      in_=ghv[:, :, 510:507:-1])

        # ---- halos (bottom 3 slots per segment), all SBUF->SBUF ----
        pstride = bandA.ap[0][0]
        for si, q in ((0, nc.sync), (1, nc.sync)):
            base = bandA.offset + si * SEG
            # band[p][slots 4..6] <- band[p+1][slots 0..2], p = 0..126
            q.dma_start(
                out=AP(bandA.tensor, base + RPP * Wp,
                       [[pstride, 127], [1, PAD * Wp]]),
                in_=AP(bandA.tensor, base + pstride,
                       [[pstride, 127], [1, PAD * Wp]]))
            # reflect edge: band[127][slot 4+t] = band[127][slot 2-t]
            q.dma_start(
                out=AP(bandA.tensor, base + 127 * pstride + RPP * Wp,
                       [[pstride, 1], [Wp, PAD], [1, Wp]]),
                in_=AP(bandA.tensor, base + 127 * pstride + 2 * Wp,
                       [[pstride, 1], [-Wp, PAD], [1, Wp]]))

        hp.__exit__(None, None, None)

        segA = bandA.rearrange("p (s r w) -> p s r w", s=2, w=Wp)
        center = segA[:, :, 0:RPP, COL0:COL0 + W]

        def nb_view(di, dj):
            return segA[:, :, di:di + RPP, COL0 + dj:COL0 + dj + W]

        sign_list = [i for i, r in enumerate(routes) if r == "sign"]
        gram_list = [i for i, r in enumerate(routes) if r == "gram"]
        p1_list = [i for i, r in enumerate(routes) if r == "p1"]
        a1_list = [i for i, r in enumerate(routes) if r == "a1"]
        n_sign = len(sign_list)

        with tc.tile_pool(name="psum", bufs=1, space="PSUM") as ppool:
            sums = ppool.tile([1, 512], dt.float32, name="sums")
            gram01 = (ppool.tile([128, 128], dt.float32, name="gram01")
                      if gram_list else None)
            sgram = ppool.tile([128, 128], dt.float32, name="sgram")
            dgram = ppool.tile([128, 128], dt.float32, name="dgram")

            sums_first = True
            g01_first = True
            sg_first = True
            dg_first = True
            # precompute which emission is the last contributor per accum
            n_sums_mm = 4 * len(a1_list) + 8 * len(gram_list)
            n_g01_mm = 16 * len(gram_list)
            n_sg_mm = 8 * len(sign_list)
            n_dg_mm = 8 * len(p1_list)
            sums_done = 0
            g01_done = 0
            sg_done = 0
            dg_done = 0

            for i, (di, dj) in enumerate(offs):
                r = routes[i]
                if r == "sign":
                    nb = nb_view(di, dj)
                    sm = {}
                    for si, nm in enumerate(("p", "t")):
                        smt = pool.tile([128, FREE], dt.float8e4,
                                        name=f"sm_{nm}_{i}", tag=f"sm_{nm}",
                                        bufs=2)
                        for h in range(2):
                            dps = ppool.tile([128, 1024], dt.float32,
                                             name=f"dps_{i}_{nm}{h}",
                                             tag="dps", bufs=2)
                            # group by lhsT (+I then -I) to halve ldweights
                            for q in range(2):
                                rr = 2 * h + q
                                nc.tensor.matmul(
                                    dps[:, q * 512:(q + 1) * 512], ident_p,
                                    center[:, si, rr:rr + 1, :],
                                    start=True, stop=False,
                                    skip_group_check=True)
                            for q in range(2):
                                rr = 2 * h + q
                                nc.tensor.matmul(
                                    dps[:, q * 512:(q + 1) * 512], ident_n,
                                    nb[:, si, rr:rr + 1, :],
                                    start=False, stop=True,
                                    skip_group_check=True)
                            nc.scalar.activation(
                                out=smt[:, h * 1024:(h + 1) * 1024],
                                in_=dps, func=AF.Sign)
                        sm[nm] = smt
                    smpv = sm["p"].rearrange("p (b j m) -> p b j m", j=2,
                                             m=128)
                    smtv = sm["t"].rearrange("p (b j m) -> p b j m", j=2,
                                             m=128)
                    for b in range(8):
                        sg_done += 1
                        nc.tensor.matmul(
                            sgram[:, :],
                            smpv[:, b], smtv[:, b],
                            start=sg_first, stop=(sg_done == n_sg_mm),
                            perf_mode=mybir.MatmulPerfMode.DoubleRow,
                            skip_group_check=True)
                        sg_first = False
                    continue

                # cmp maps; the first few offsets split per-image so the
                # pred side runs while target channels are still loading
                cmp = pool.tile([128, PAIR], dt.bfloat16,
                                name=f"cmp_{i}", tag=f"cmp_{r}",
                                bufs=(5 if r == "p1" else 3))
                cv = cmp.rearrange("p (s r w) -> p s r w", s=2, w=W)
                nbv = nb_view(di, dj)
                if di == 0 and i < 3:
                    # di==0 rows r only read band slots r: the top half is
                    # ready right after the first half-gray of each image
                    for si in range(2):
                        for r0, r1 in ((0, 2), (2, RPP)):
                            nc.vector.tensor_tensor(
                                out=cv[:, si:si + 1, r0:r1],
                                in0=center[:, si:si + 1, r0:r1],
                                in1=nbv[:, si:si + 1, r0:r1],
                                op=op.is_gt)
                elif i < 10:
                    for si in range(2):
                        nc.vector.tensor_tensor(out=cv[:, si:si + 1],
                                                in0=center[:, si:si + 1],
                                                in1=nbv[:, si:si + 1],
                                                op=op.is_gt)
                else:
                    nc.vector.tensor_tensor(out=cv, in0=center, in1=nbv,
                                            op=op.is_gt)

                if r == "p1":
                    # d = cmpP - cmpT in {-1,0,1} (fp8 exact); XOR = sum(d^2)
                    dd = pool.tile([128, FREE], dt.float8e4,
                                   name=f"dd_{i}", tag="dd", bufs=4)
                    nc.gpsimd.tensor_tensor(out=dd, in0=cmp[:, 0:FREE],
                                            in1=cmp[:, FREE:PAIR],
                                            op=op.subtract)
                    ddv = dd.rearrange("p (b j m) -> p b j m", j=2, m=128)
                    for b in range(8):
                        dg_done += 1
                        nc.tensor.matmul(
                            dgram[:, :], ddv[:, b], ddv[:, b],
                            start=dg_first, stop=(dg_done == n_dg_mm),
                            perf_mode=mybir.MatmulPerfMode.DoubleRow,
                            skip_group_check=True)
                        dg_first = False
                elif r == "a1":
                    xo = pool.tile([128, FREE], dt.bfloat16,
                                   name=f"xo_{i}", tag="xo", bufs=4)
                    nc.vector.tensor_tensor(out=xo, in0=cmp[:, 0:FREE],
                                            in1=cmp[:, FREE:PAIR],
                                            op=op.not_equal)
                    for k in range(4):
                        sums_done += 1
                        nc.tensor.matmul(
                            sums[0:1, :], ones_bf[:, 0:1],
                            xo[:, k * 512:(k + 1) * 512],
                            start=sums_first, stop=(sums_done == n_sums_mm),
                            skip_group_check=True)
                        sums_first = False
                else:  # gram
                    for b in range(16):
                        g01_done += 1
                        nc.tensor.matmul(
                            gram01[:, :],
                            cmp[:, b * 128:(b + 1) * 128],
                            cmp[:, FREE + b * 128:FREE + (b + 1) * 128],
                            start=g01_first, stop=(g01_done == n_g01_mm),
                            skip_group_check=True)
                        g01_first = False
                    for k in range(8):
                        sums_done += 1
                        nc.tensor.matmul(
                            sums[0:1, :], ones_bf[:, 0:1],
                            cmp[:, k * 512:(k + 1) * 512],
                            start=sums_first, stop=(sums_done == n_sums_mm),
                            skip_group_check=True)
                        sums_first = False

            # ---- drain accumulators to SBUF and DMA out ----
            sums_sb = pool.tile([1, 512], dt.float32, name="sums_sb",
                                tag="sums_sb")
            g01_sb = pool.tile([128, 128], dt.float32, name="g01_sb",
                               tag="g01_sb")
            sg_sb = pool.tile([128, 128], dt.float32, name="sg_sb",
                              tag="sg_sb")
            dg_sb = pool.tile([128, 128], dt.float32, name="dg_sb",
                              tag="dg_sb")
            # drain order: accumulators that stop earliest first, so their
            # output DMAs don't queue behind the late-stopping sums
            if n_dg_mm:
                nc.vector.tensor_copy(out=dg_sb, in_=dgram)
                nc.sync.dma_start(out=dgram_out.ap(), in_=dg_sb)
            if n_sg_mm:
                nc.vector.tensor_copy(out=sg_sb, in_=sgram)
                nc.sync.dma_start(out=sgram_out.ap(), in_=sg_sb)
            if n_g01_mm:
                nc.vector.tensor_copy(out=g01_sb, in_=gram01)
                nc.sync.dma_start(out=gram01_out.ap(), in_=g01_sb)
            if n_sums_mm:
                nc.vector.tensor_copy(out=sums_sb, in_=sums)
                nc.sync.dma_start(out=sums_out.ap(), in_=sums_sb)
            nc._use_outs = (n_sums_mm > 0, n_g01_mm > 0, n_sg_mm > 0,
                            n_dg_mm > 0)

    nc.finalize()
    nc._n_sign = n_sign
    return nc


def kernel(pred: np.ndarray, target: np.ndarray) -> np.ndarray:
    from concourse import bass_utils

    if "nc" not in _CACHE:
        _CACHE["nc"] = _build_bass()
    nc = _CACHE["nc"]
    n_sign = nc._n_sign

    pred = np.ascontiguousarray(pred, dtype=np.float32)
    target = np.ascontiguousarray(target, dtype=np.float32)
    in_maps = [
        {"pred": pred[b], "target": target[b]} for b in range(N_CORES)
    ]
    res = bass_utils.run_bass_kernel_spmd(nc, in_maps,
                                          core_ids=list(range(N_CORES)))
    use_sums, use_g01, use_sg, use_dg = nc._use_outs
    total = 0.0
    for r in res.results:
        if use_sums:
            total += float(r["sums_out"].astype(np.float64).sum())
        if use_g01:
            total -= 2.0 * float(
                np.diag(r["gram01_out"]).astype(np.float64).sum())
        if use_dg:
            total += float(np.diag(r["dgram_out"]).astype(np.float64).sum())
        if use_sg:
            total += (n_sign * NPIX - float(
                np.diag(r["sgram_out"]).astype(np.float64).sum())) / 2.0
    mean = 2.0 * total / (B * 48 * H * W)
    return np.array(mean, dtype=np.float32)
